# revision 1
# baseline (speedup 1.0000x reference)
"""Distributed contrastive loss (nn_ContrastiveLoss) as a Trainium2 Bass kernel.

Shapes are hardcoded: B=32, T=D=256, f32. 8 NeuronCores, data-parallel over
the anchor index i (4 anchors per core); every core receives the full
back_VF/back_AF (the "all-gather" is done host-side by replicating inputs)
plus its own 4-row shard of each.

Math per direction (V, A):
  rows[i,t,s] = log(1 + sum_{j != i} exp(<V_i[t], A_j[s]> / (||V_i||_F * ||A_j[:,s]||)))
Output = -(rows_V + rows_A) reshaped to [B*T, T].

Kernel layout choice: per-(i,j) product tile is [s(part), t(free)] so the
1/a_norm[j,s] factor is a per-partition activation scale fused into Exp.
1/v_norm[i] is folded into the V operand. The j-sum runs on the tensor
engine as identity-matmul PSUM accumulation of bf16 exp tiles; j==i is
removed by a negated-identity matmul of the separately computed diagonal
tile (bit-identical data path, so it cancels exactly).
"""

import numpy as np
import ml_dtypes

import concourse.bacc as bacc
import concourse.tile as tile
from concourse import mybir

FP32 = mybir.dt.float32
BF16 = mybir.dt.bfloat16
AFT = mybir.ActivationFunctionType
ALU = mybir.AluOpType

B, T, D = 32, 256, 256
NCORES = 8
SH = B // NCORES          # 4 anchors per core
EPS = 1e-18
BIAS = 1.0

_COMPILED = None  # (nc, out_name) cache


def _build():
    nc = bacc.Bacc("TRN2", target_bir_lowering=False, debug=False,
                   num_devices=NCORES)

    vf = nc.dram_tensor("vf", [B, T, D], FP32, kind="ExternalInput").ap()
    af = nc.dram_tensor("af", [B, T, D], FP32, kind="ExternalInput").ap()
    vfs = nc.dram_tensor("vfs", [SH, T, D], FP32, kind="ExternalInput").ap()
    afs = nc.dram_tensor("afs", [SH, T, D], FP32, kind="ExternalInput").ap()
    idbd = nc.dram_tensor("idb", [128, 128], BF16, kind="ExternalInput").ap()
    nidbd = nc.dram_tensor("nidb", [128, 128], BF16, kind="ExternalInput").ap()
    idfd = nc.dram_tensor("idf", [128, 128], FP32, kind="ExternalInput").ap()
    onesd = nc.dram_tensor("onesf", [128, 128], FP32, kind="ExternalInput").ap()
    out = nc.dram_tensor("out", [SH * T, T], FP32, kind="ExternalOutput").ap()

    with tile.TileContext(nc) as tc:
        with (
            tc.tile_pool(name="const", bufs=1) as constp,
            tc.tile_pool(name="res", bufs=1) as resp,
            tc.tile_pool(name="nat", bufs=3) as natp,
            tc.tile_pool(name="work", bufs=2) as workp,
            tc.tile_pool(name="ep", bufs=3) as ep_,
            tc.tile_pool(name="psA", bufs=4, space="PSUM") as psA,
            tc.tile_pool(name="psB", bufs=2, space="PSUM") as psB,
            tc.tile_pool(name="psC", bufs=2, space="PSUM") as psC,
        ):
            # ---- constants ----
            idb = constp.tile([128, 128], BF16, tag="idb")
            nidb = constp.tile([128, 128], BF16, tag="nidb")
            idf = constp.tile([128, 128], FP32, tag="idf")
            ones = constp.tile([128, 128], FP32, tag="ones")
            nc.sync.dma_start(idb[:], idbd[:])
            nc.sync.dma_start(nidb[:], nidbd[:])
            nc.sync.dma_start(idf[:], idfd[:])
            nc.sync.dma_start(ones[:], onesd[:])

            # ---- resident tiles ----
            # transposed bf16 copies: matrix j occupies [:, j*512:(j+1)*512];
            # within that, free = a*256 + r (a = original column half,
            # partition p = column index within half a, r = original row).
            ttvf = resp.tile([128, B * 512], BF16, tag="ttvf")
            ttaf = resp.tile([128, B * 512], BF16, tag="ttaf")
            ttvfs = resp.tile([128, SH * 512], BF16, tag="ttvfs")
            ttafs = resp.tile([128, SH * 512], BF16, tag="ttafs")
            # V-role operands with 1/v_norm folded in:
            # [:, dir*2048 + b*1024 + a*512 + m*256 + t]
            vpair = resp.tile([128, 2 * 2 * 1024], BF16, tag="vpair")
            # diagonal exp tiles: [:, ((dir*SH + k)*2 + sh)*256 + t]
            eii = resp.tile([128, 2 * SH * 2 * 256], BF16, tag="eii")
            # dir0 log rows: [:, b*1024 + sh*512 + ...]
            rows0 = resp.tile([128, 2 * 1024], FP32, tag="rows0")
            # column sum-squares (an2) and rsqrt tiles
            an2f = [resp.tile([128, 64], FP32, tag=f"an2f{i}", name=f"an2f{i}")
                    for i in range(2)]
            an2s = [resp.tile([128, 8], FP32, tag=f"an2s{i}", name=f"an2s{i}")
                    for i in range(2)]
            recf = [resp.tile([128, 64], FP32, tag=f"recf{i}", name=f"recf{i}")
                    for i in range(2)]
            recs = [resp.tile([128, 8], FP32, tag=f"recs{i}", name=f"recs{i}")
                    for i in range(2)]
            v2row = resp.tile([1, 2 * SH], FP32, tag="v2row")
            invsb = resp.tile([1, 2 * SH], FP32, tag="invsb")
            invb = resp.tile([128, 2 * SH], FP32, tag="invb")

            def load_transpose(src_ap, j, tt, an2, col):
                """DMA matrix j, downcast, transpose via identity matmul,
                stash bf16 transposed copy, accumulate column sum-squares."""
                nat32 = natp.tile([128, 512], FP32, tag="nat32")
                nc.sync.dma_start(nat32[:, 0:256], src_ap[j, 0:128, :])
                nc.sync.dma_start(nat32[:, 256:512], src_ap[j, 128:256, :])
                nat16 = natp.tile([128, 512], BF16, tag="nat16")
                nc.vector.tensor_copy(nat16[:], nat32[:])
                ps = psC.tile([128, 512], FP32, tag="tp")
                for u in range(2):          # original row half
                    for a in range(2):      # original column half
                        nc.tensor.matmul(
                            ps[:, a * 256 + u * 128:a * 256 + u * 128 + 128],
                            nat16[:, u * 256 + a * 128:u * 256 + a * 128 + 128],
                            idb[:],
                            start=True, stop=True)
                dst = tt[:, j * 512:(j + 1) * 512]
                nc.vector.tensor_copy(dst, ps[:])
                # column sum-squares via ACT Square + accum_out (free-axis
                # sum). DVE reduce-class ops fail at execute on this axon
                # path; activation accum_out is the proven alternative.
                sq = workp.tile([128, 512], FP32, tag="sq")
                for a in range(2):
                    nc.scalar.activation(
                        sq[:, a * 256:(a + 1) * 256],
                        dst[:, a * 256:(a + 1) * 256], AFT.Square,
                        accum_out=an2[:, col + a:col + a + 1])

            def rsqrt(dst, src, n, p=128):
                """dst = 1/sqrt(src) via exp(-0.5*ln(x)) — stays in the
                natural_log_exp table set (no ACT table reloads). The
                reference's +EPS=1e-18 is sub-ULP against an2 >= ~180 in
                fp32 (256 + 1e-18 == 256), so bias=0 is bit-identical."""
                t = workp.tile([128, 64], FP32, tag="lnt")
                nc.scalar.activation(t[0:p, 0:n], src, AFT.Ln, bias=0.0)
                nc.scalar.activation(dst, t[0:p, 0:n], AFT.Exp, scale=-0.5)

            # ---- shard prologue ----
            for i, (src, tt, an2) in enumerate(
                    ((vfs, ttvfs, an2s[0]), (afs, ttafs, an2s[1]))):
                for k in range(SH):
                    load_transpose(src, k, tt, an2, 2 * k)

            # v2[dir,k] = total sum-square of the dir's V-role shard matrix
            v2p = psC.tile([1, 2 * SH], FP32, tag="tp")
            for dr in range(2):
                a2 = an2s[dr]  # dir0 V-role = VF shard, dir1 = AF shard
                for k in range(SH):
                    for a in range(2):
                        nc.tensor.matmul(
                            v2p[0:1, dr * SH + k:dr * SH + k + 1],
                            ones[:, 0:1], a2[:, 2 * k + a:2 * k + a + 1],
                            start=(a == 0), stop=(a == 1))
            nc.vector.tensor_copy(v2row[:], v2p[:])
            rsqrt(invsb[0:1, 0:2 * SH], v2row[0:1, 0:2 * SH], 2 * SH, p=1)
            # broadcast each 1/v_norm over 128 partitions
            invbp = psC.tile([128, 2 * SH], FP32, tag="tp")
            for c in range(2 * SH):
                nc.tensor.matmul(invbp[:, c:c + 1], ones[0:1, 0:128],
                                 invsb[0:1, c:c + 1], start=True, stop=True)
            nc.vector.tensor_copy(invb[:], invbp[:])

            # rsqrt of shard an2 (for the diagonal tiles' exp scale):
            # dir0 diagonal A-role = AF shard, dir1 = VF shard
            rsqrt(recs[0][:, 0:8], an2s[1][:, 0:8], 8)
            rsqrt(recs[1][:, 0:8], an2s[0][:, 0:8], 8)

            # vpair: V-role transposed operands scaled by 1/v_norm
            for dr, tts in enumerate((ttvfs, ttafs)):
                for k in range(SH):
                    b, m = k // 2, k % 2
                    for a in range(2):
                        nc.vector.tensor_scalar_mul(
                            vpair[:, dr * 2048 + b * 1024 + a * 512 + m * 256:
                                  dr * 2048 + b * 1024 + a * 512 + m * 256 + 256],
                            tts[:, k * 512 + a * 256:k * 512 + (a + 1) * 256],
                            invb[:, dr * SH + k:dr * SH + k + 1])

            # ---- diagonal tiles e_ii = exp(sim(i,i)) ----
            for dr, tts_a in enumerate((ttafs, ttvfs)):
                for k in range(SH):
                    b, m = k // 2, k % 2
                    pii = psA.tile([128, 512], FP32, tag="prod")
                    for sh in range(2):
                        for a in range(2):
                            nc.tensor.matmul(
                                pii[:, sh * 256:(sh + 1) * 256],
                                tts_a[:, k * 512 + a * 256 + sh * 128:
                                      k * 512 + a * 256 + sh * 128 + 128],
                                vpair[:, dr * 2048 + b * 1024 + a * 512 + m * 256:
                                      dr * 2048 + b * 1024 + a * 512 + m * 256 + 256],
                                start=(a == 0), stop=(a == 1))
                    for sh in range(2):
                        nc.scalar.activation(
                            eii[:, ((dr * SH + k) * 2 + sh) * 256:
                                ((dr * SH + k) * 2 + sh) * 256 + 256],
                            pii[:, sh * 256:(sh + 1) * 256], AFT.Exp,
                            scale=recs[dr][:, 2 * k + sh:2 * k + sh + 1])

            # ---- full-tensor prologue + main loop, per direction ----
            def full_prologue(src, tt, an2, rec):
                for j in range(B):
                    load_transpose(src, j, tt, an2, 2 * j)
                    if j % 8 == 7:  # rsqrt in chunks of 8 matrices
                        c0 = (j - 7) * 2
                        rsqrt(rec[:, c0:c0 + 16], an2[:, c0:c0 + 16], 16)

            def main_direction(dr, tt_a, rec_a):
                for b in range(2):
                    acc = [psB.tile([128, 512], FP32, tag="acc", name=f"acc{sh}")
                           for sh in range(2)]
                    for j in range(B):
                        prod = [psA.tile([128, 512], FP32, tag="prod",
                                         name=f"prod{sh}")
                                for sh in range(2)]
                        for sh in range(2):
                            for a in range(2):
                                nc.tensor.matmul(
                                    prod[sh][:],
                                    tt_a[:, j * 512 + a * 256 + sh * 128:
                                         j * 512 + a * 256 + sh * 128 + 128],
                                    vpair[:, dr * 2048 + b * 1024 + a * 512:
                                          dr * 2048 + b * 1024 + (a + 1) * 512],
                                    start=(a == 0), stop=(a == 1))
                        for sh in range(2):
                            e = ep_.tile([128, 512], BF16, tag="e")
                            nc.scalar.activation(
                                e[:], prod[sh][:], AFT.Exp,
                                scale=rec_a[:, 2 * j + sh:2 * j + sh + 1])
                            nc.tensor.matmul(acc[sh][:], idb[:], e[:],
                                             start=(j == 0), stop=False,
                                             skip_group_check=True)
                    # subtract the j==i diagonal tile
                    for sh in range(2):
                        for m in range(2):
                            k = b * 2 + m
                            nc.tensor.matmul(
                                acc[sh][:, m * 256:(m + 1) * 256], nidb[:],
                                eii[:, ((dr * SH + k) * 2 + sh) * 256:
                                    ((dr * SH + k) * 2 + sh) * 256 + 256],
                                start=False, stop=(m == 1),
                                skip_group_check=True)
                    # rows = log(1 + acc)
                    if dr == 0:
                        for sh in range(2):
                            nc.scalar.activation(
                                rows0[:, b * 1024 + sh * 512:
                                      b * 1024 + (sh + 1) * 512],
                                acc[sh][:], AFT.Ln, bias=1.0)
                    else:
                        combs = []
                        for sh in range(2):
                            r1 = ep_.tile([128, 512], FP32, tag="r1")
                            nc.scalar.activation(r1[:], acc[sh][:], AFT.Ln,
                                                 bias=1.0)
                            comb = workp.tile([128, 512], FP32, tag="comb")
                            nc.vector.tensor_add(
                                comb[:], r1[:],
                                rows0[:, b * 1024 + sh * 512:
                                      b * 1024 + (sh + 1) * 512])
                            combs.append(comb)
                        # transpose [s,t] -> [t,s], negate on PSUM->SBUF copy
                        for m in range(2):
                            k = b * 2 + m
                            for u in range(2):
                                ot = psC.tile([128, 256], FP32, tag="tp")
                                for sh in range(2):
                                    nc.tensor.matmul(
                                        ot[:, sh * 128:(sh + 1) * 128],
                                        combs[sh][:, m * 256 + u * 128:
                                                   m * 256 + u * 128 + 128],
                                        idf[:], start=True, stop=True)
                                ost = ep_.tile([128, 256], FP32, tag="ost")
                                nc.vector.tensor_scalar_mul(ost[:], ot[:], -1.0)
                                nc.sync.dma_start(
                                    out[k * 256 + u * 128:
                                        k * 256 + u * 128 + 128, :], ost[:])

            full_prologue(af, ttaf, an2f[0], recf[0])   # dir0 A-role = AF
            main_direction(0, ttaf, recf[0])
            full_prologue(vf, ttvf, an2f[1], recf[1])   # dir1 A-role = VF
            main_direction(1, ttvf, recf[1])

    nc.compile()
    return nc


def _consts():
    eye32 = np.eye(128, dtype=np.float32)
    return {
        "idb": eye32.astype(ml_dtypes.bfloat16),
        "nidb": (-eye32).astype(ml_dtypes.bfloat16),
        "idf": eye32,
        "onesf": np.ones((128, 128), np.float32),
    }


def kernel(**inputs):
    global _COMPILED
    from concourse.bass_utils import run_bass_kernel_spmd

    VF = np.ascontiguousarray(np.asarray(inputs["back_VF"], np.float32))
    AF = np.ascontiguousarray(np.asarray(inputs["back_AF"], np.float32))

    if _COMPILED is None:
        _COMPILED = _build()
    nc = _COMPILED

    consts = _consts()
    in_maps = []
    for c in range(NCORES):
        in_maps.append({
            "vf": VF, "af": AF,
            "vfs": np.ascontiguousarray(VF[c * SH:(c + 1) * SH]),
            "afs": np.ascontiguousarray(AF[c * SH:(c + 1) * SH]),
            **consts,
        })
    res = run_bass_kernel_spmd(nc, in_maps, core_ids=list(range(NCORES)))
    full = np.concatenate([res.results[c]["out"] for c in range(NCORES)],
                          axis=0)
    return (1.0 / BIAS) * full  # negation already applied on-device

